# revision 1
# baseline (speedup 1.0000x reference)
"""Sparse neighbor-attention (point transformer style) on 8 Trainium2 cores.

Strategy (segment/data parallel, per sharding hint):
- Points are sharded contiguously: core c owns points [c*6250, (c+1)*6250).
- The small qkv/proj weights are replicated (host pre-transposes / pre-scales).
- Every core computes the full k|v table [50048, 512] bf16 in its own HBM
  (neighbor ids are uniform random, so every core needs all rows; replicated
  compute on the idle-ish PE beats a collective here and keeps cores
  independent).
- Per 128-point tile, one SWDGE indirect DMA gathers the 16 neighbor k|v rows
  per point; attention arithmetic runs in fp32 on DVE/ACT; the output
  projection contracts channels on the PE after an on-chip transpose.

Self-contained: builds the Bass program, shards inputs on the host, runs via
run_bass_kernel_spmd on cores 0-7, reassembles the full [50000, 256] output.
"""
import math
import os
import sys
from contextlib import ExitStack

import numpy as np

for _p in ('/opt/trn_rl_repo', '/root/.axon_site/_ro/trn_rl_repo'):
    if os.path.isdir(_p) and _p not in sys.path:
        sys.path.append(_p)

import ml_dtypes
import concourse.bass as bass
import concourse.mybir as mybir
import concourse.tile as tile
from concourse.masks import make_identity
from concourse.bass_utils import run_bass_kernel_spmd

# ---------------------------------------------------------------------------
# Workaround: this container's walrus rejects >2 sync waits on one
# instruction ("Too many sync wait commands" in setupSyncWait). Split excess
# waits onto same-engine nops committed immediately before the instruction.
_MAX_WAITS = 1
_orig_commit = tile.TileContext._commit_instruction


def _commit_split_waits(self, inst, lazy_reg_writes=True):
    si = getattr(inst, "sync_info", None)
    if si is not None and len(si.on_wait) > _MAX_WAITS:
        waits = list(si.on_wait)
        keep = waits[:_MAX_WAITS]
        rest = waits[_MAX_WAITS:]
        si.on_wait.clear()
        for w in keep:
            si.on_wait.append(w)
        for i in range(0, len(rest), _MAX_WAITS):
            nop = mybir.InstNoOp(
                name=self.nc.get_next_instruction_name(),
                engine=inst.engine,
                bass_nofuse=True,
                sync_info=mybir.SyncInfo(
                    on_wait=rest[i:i + _MAX_WAITS], on_update=[]),
            )
            _orig_commit(self, nop, lazy_reg_writes=False)
    return _orig_commit(self, inst, lazy_reg_writes=lazy_reg_writes)


tile.TileContext._commit_instruction = _commit_split_waits


def _drain_and_barrier_split(self, tick_clock, wait_clock):
    import bass_rust as _br
    carrier = self.nc.sync.nop(nofuse=True, hint="drain_wait_carrier")
    wait_clock.add_sem_waits(carrier.ins,
                             _br.ScopedClock({None: tick_clock.global_clock}))
    si = carrier.ins.sync_info
    waits = list(si.on_wait) if si is not None else []
    if si is not None:
        si.on_wait.clear()
    for w in waits:
        nop = self.nc.sync.nop(nofuse=True, hint="drain_wait_split")
        nsi = nop.ins.sync_info
        if nsi is None:
            nop.ins.sync_info = mybir.SyncInfo(on_wait=[w], on_update=[])
        else:
            nsi.on_wait.append(w)
    self.nc.sync.drain()
    self.nc.all_engine_barrier()
    assert self.sems is not None
    popped = self.nc._tile_sem_poison_stack.pop()
    assert popped is self._sem_poison
    self.nc.clear_and_free_semaphores(list(self.sems.allocated().values()))
    self.nc.all_engine_barrier()


tile.TileContext._drain_and_barrier = _drain_and_barrier_split
# ---------------------------------------------------------------------------

P = 128
F32 = mybir.dt.float32
BF16 = mybir.dt.bfloat16
I32 = mybir.dt.int32
ALU = mybir.AluOpType
AXT = mybir.AxisListType
ACTF = mybir.ActivationFunctionType

N_CORES = 8
N_TOTAL = 50000
K = 16
DIM = 256
H = 8
HD = DIM // H

LAST_EXEC_NS = None
_PROGRAM_CACHE = {}


def _bcast_ap(ap, insert_axis, count):
    dims = list(ap.ap)
    dims.insert(insert_axis, [0, count])
    return bass.AP(ap.tensor, ap.offset, dims)


def _build(n_total, n_own):
    TA = math.ceil(n_total / P)
    NPAD = TA * P
    TO = math.ceil(n_own / P)
    NOWN_PAD = TO * P
    D2 = 2 * DIM

    nc = bass.Bass()
    featsT = nc.dram_tensor("featsT", [DIM, NPAD], BF16, kind="ExternalInput")
    fownT = nc.dram_tensor("fownT", [DIM, NOWN_PAD], BF16, kind="ExternalInput")
    wkvT = nc.dram_tensor("wkvT", [DIM, D2], BF16, kind="ExternalInput")
    wqT = nc.dram_tensor("wqT", [DIM, DIM], BF16, kind="ExternalInput")
    bq = nc.dram_tensor("bq", [1, DIM], BF16, kind="ExternalInput")
    wpT = nc.dram_tensor("wpT", [DIM, DIM], BF16, kind="ExternalInput")
    bp = nc.dram_tensor("bp", [1, DIM], BF16, kind="ExternalInput")
    idx = nc.dram_tensor("idx", [P, TO, K], I32, kind="ExternalInput")
    kv = nc.dram_tensor("kv", [NPAD, D2], BF16, kind="Internal")
    out = nc.dram_tensor("out", [NOWN_PAD, DIM], F32, kind="ExternalOutput")

    with tile.TileContext(nc) as tc, ExitStack() as ctx:
        singles = ctx.enter_context(tc.tile_pool(name="singles", bufs=1))
        fpool = ctx.enter_context(tc.tile_pool(name="fpool", bufs=3))
        kpool = ctx.enter_context(tc.tile_pool(name="kpool", bufs=3))
        gpool = ctx.enter_context(tc.tile_pool(name="gpool", bufs=3))
        cpool = ctx.enter_context(tc.tile_pool(name="cpool", bufs=2))
        cpool3 = ctx.enter_context(tc.tile_pool(name="cpool3", bufs=3))
        opool = ctx.enter_context(tc.tile_pool(name="opool", bufs=2))
        psum = ctx.enter_context(tc.tile_pool(name="psum", bufs=2, space="PSUM"))

        w_kv = singles.tile([P, 2, D2], BF16)
        nc.sync.dma_start(out=w_kv[:], in_=wkvT[:, :].rearrange("(b p) m -> p b m", p=P))
        w_q = singles.tile([P, 2, DIM], BF16)
        nc.sync.dma_start(out=w_q[:], in_=wqT[:, :].rearrange("(b p) m -> p b m", p=P))
        w_p = singles.tile([P, 2, DIM], BF16)
        nc.sync.dma_start(out=w_p[:], in_=wpT[:, :].rearrange("(b p) m -> p b m", p=P))
        b_q = singles.tile([1, DIM], BF16)
        nc.sync.dma_start(out=b_q[:], in_=bq[:, :])
        b_p = singles.tile([1, DIM], BF16)
        nc.sync.dma_start(out=b_p[:], in_=bp[:, :])
        idx_all = singles.tile([P, TO, K], I32)
        nc.sync.dma_start(out=idx_all[:], in_=idx[:, :, :])
        ones = singles.tile([1, P], BF16)
        nc.vector.memset(ones[:], 1.0)
        ident = singles.tile([P, P], BF16)
        make_identity(nc, ident[:])
        q_all = singles.tile([P, TO, DIM], BF16)

        # ---- phase A: k|v table ------------------------------------------
        # Biases are not needed here: the k bias is constant over a point's
        # neighbors so it cancels in the softmax, and the v bias is folded
        # into the projection bias on the host (softmax weights sum to 1).
        # Two point-tiles per iteration to halve DMA-issue overhead; casts on
        # the (otherwise idle) DVE; stores issued from ACT's HWDGE queue.
        for st in range(math.ceil(TA / 2)):
            t0s = 2 * st
            nt = min(2, TA - t0s)
            ft = fpool.tile([P, 2, 2 * P], BF16, tag="ft")
            nc.sync.dma_start(
                out=ft[:, :, 0:nt * P],
                in_=featsT[:, t0s * P:(t0s + nt) * P]
                .rearrange("(b p) i -> p b i", p=P))
            kvsb = kpool.tile([P, 2, D2], BF16, tag="kvsb")
            for u in range(nt):
                kvps = psum.tile([P, D2], F32, tag="kvps", bufs=3)
                nc.tensor.matmul(out=kvps[:], lhsT=ft[:, 0, u * P:(u + 1) * P],
                                 rhs=w_kv[:, 0, :], start=True, stop=False)
                nc.tensor.matmul(out=kvps[:], lhsT=ft[:, 1, u * P:(u + 1) * P],
                                 rhs=w_kv[:, 1, :], start=False, stop=True)
                nc.vector.tensor_copy(out=kvsb[:, u, :], in_=kvps[:])
            nc.scalar.dma_start(
                out=kv[t0s * P:(t0s + nt) * P, :]
                .rearrange("(u p) m -> p u m", p=P),
                in_=kvsb[:, 0:nt, :])

        # ---- phase B: q for own points -----------------------------------
        for tb in range(TO):
            fo = fpool.tile([P, 2, P], BF16, tag="fo")
            nc.sync.dma_start(
                out=fo[:],
                in_=fownT[:, tb * P:(tb + 1) * P].rearrange("(b p) i -> p b i", p=P))
            qps = psum.tile([P, DIM], F32, tag="qps", bufs=1)
            nc.tensor.matmul(out=qps[:], lhsT=fo[:, 0, :], rhs=w_q[:, 0, :],
                             start=True, stop=False)
            nc.tensor.matmul(out=qps[:], lhsT=fo[:, 1, :], rhs=w_q[:, 1, :],
                             start=False, stop=False)
            nc.tensor.matmul(out=qps[:], lhsT=ones[:1, :], rhs=b_q[:1, :],
                             start=False, stop=True)
            nc.scalar.copy(out=q_all[:, tb, :], in_=qps[:])

        # ---- phase C: attention + projection -----------------------------
        for t in range(TO):
            kvg = gpool.tile([P, K, D2], BF16, tag="kvg")
            # One [P,1] indirect DMA per neighbor slot: the HW vector-indirect
            # consumes one offset per descriptor (per dest partition run), so
            # only the one-row-per-partition form gathers correctly.
            for j in range(K):
                nc.gpsimd.indirect_dma_start(
                    out=kvg[:, j, :], out_offset=None, in_=kv[:, :],
                    in_offset=bass.IndirectOffsetOnAxis(
                        ap=idx_all[:, t, j:j + 1], axis=0))
            prod = cpool3.tile([P, K, DIM], BF16, tag="prod")
            qb = q_all[:, t, :]
            nc.vector.tensor_tensor(out=prod[:], in0=kvg[:, :, 0:DIM],
                                    in1=_bcast_ap(qb, 1, K), op=ALU.mult)
            # d-reduction as a bf16 add tree (2x mode) + final fp32 reduce
            pv = prod[:].rearrange("p k (h x) -> p (k h) x", h=H)  # [P,128,32]
            r1 = cpool.tile([P, K * H, 16], BF16, tag="r1")
            nc.vector.tensor_tensor(out=r1[:], in0=pv[:, :, 0:16],
                                    in1=pv[:, :, 16:32], op=ALU.add)
            r2 = cpool.tile([P, K * H, 8], BF16, tag="r2")
            nc.vector.tensor_tensor(out=r2[:], in0=r1[:, :, 0:8],
                                    in1=r1[:, :, 8:16], op=ALU.add)
            scores = cpool.tile([P, K * H], F32, tag="scores")
            nc.vector.tensor_reduce(out=scores[:], in_=r2[:],
                                    axis=AXT.X, op=ALU.add)
            # no max-subtraction: scores are O(|q||k|) ~ +-8 here, exp is
            # fp32-safe, and softmax is shift-invariant so results match
            ex = cpool.tile([P, K * H], F32, tag="ex")
            nc.scalar.activation(out=ex[:], in_=scores[:], func=ACTF.Exp)
            den = cpool.tile([P, H], F32, tag="den")
            nc.vector.tensor_reduce(
                out=den[:], in_=ex[:].rearrange("p (k h) -> p h k", h=H),
                axis=AXT.X, op=ALU.add)
            rec = cpool.tile([P, H], F32, tag="rec")
            nc.vector.reciprocal(rec[:], den[:])
            # expand raw exp weights over head-dim on ACT (frees DVE); the
            # 1/denominator is applied once after the k-reduction
            aexp = cpool3.tile([P, K, DIM], BF16, tag="aexp")
            aw4 = _bcast_ap(ex[:].rearrange("p (k h) -> p k h", h=H), 3, HD)
            nc.scalar.copy(
                out=aexp[:].rearrange("p k (h d) -> p k h d", h=H), in_=aw4)
            prod2 = cpool.tile([P, K, DIM], BF16, tag="prod2")
            nc.vector.tensor_tensor(out=prod2[:], in0=kvg[:, :, DIM:D2],
                                    in1=aexp[:], op=ALU.mult)
            s1 = cpool.tile([P, 8, DIM], BF16, tag="s1")
            nc.gpsimd.tensor_tensor(out=s1[:], in0=prod2[:, 0:8, :],
                                    in1=prod2[:, 8:16, :], op=ALU.add)
            s2 = cpool.tile([P, 4, DIM], BF16, tag="s2")
            nc.vector.tensor_tensor(out=s2[:], in0=s1[:, 0:4, :],
                                    in1=s1[:, 4:8, :], op=ALU.add)
            xout = cpool.tile([P, DIM], F32, tag="xout")
            nc.vector.tensor_reduce(
                out=xout[:], in_=s2[:].rearrange("p k c -> p c k"),
                axis=AXT.X, op=ALU.add)
            # normalize by 1/den fused with the bf16 downcast
            xbf = cpool.tile([P, DIM], BF16, tag="xbf")
            nc.vector.tensor_tensor(
                out=xbf[:].rearrange("p (h d) -> p h d", h=H),
                in0=xout[:].rearrange("p (h d) -> p h d", h=H),
                in1=_bcast_ap(rec[:], 2, HD), op=ALU.mult)
            xT = opool.tile([P, 2, P], BF16, tag="xT")
            for b in range(2):
                tps = psum.tile([P, P], BF16, tag="tps")
                nc.tensor.transpose(out=tps[:], in_=xbf[:, b * P:(b + 1) * P],
                                    identity=ident[:])
                nc.scalar.copy(out=xT[:, b, :], in_=tps[:])
            pps = psum.tile([P, DIM], F32, tag="pps")
            nc.tensor.matmul(out=pps[:], lhsT=xT[:, 0, :], rhs=w_p[:, 0, :],
                             start=True, stop=False)
            nc.tensor.matmul(out=pps[:], lhsT=xT[:, 1, :], rhs=w_p[:, 1, :],
                             start=False, stop=False)
            nc.tensor.matmul(out=pps[:], lhsT=ones[:1, :], rhs=b_p[:1, :],
                             start=False, stop=True)
            osb = opool.tile([P, DIM], F32, tag="osb")
            nc.scalar.copy(out=osb[:], in_=pps[:])
            nc.sync.dma_start(out=out[t * P:(t + 1) * P, :], in_=osb[:])

    nc.finalize()
    return nc


def _host_prep(feats, index_1, qkv_w, qkv_b, proj_w, proj_b):
    bf16 = ml_dtypes.bfloat16
    N = feats.shape[0]
    scale = HD ** -0.5
    TA = math.ceil(N / P)
    NPAD = TA * P
    n_own = N // N_CORES
    TO = math.ceil(n_own / P)
    NOWN_PAD = TO * P

    featsT = np.zeros((DIM, NPAD), dtype=bf16)
    featsT[:, :N] = np.asarray(feats, dtype=np.float32).T.astype(bf16)
    qkv_w = np.asarray(qkv_w, dtype=np.float32)
    qkv_b = np.asarray(qkv_b, dtype=np.float32)
    wqT = np.ascontiguousarray((qkv_w[0:DIM] * scale).astype(bf16).T)
    bqv = (qkv_b[0:DIM] * scale).astype(bf16).reshape(1, -1)
    wkvT = np.ascontiguousarray(qkv_w[DIM:3 * DIM].astype(bf16).T)
    proj_w = np.asarray(proj_w, np.float32)
    wpT = np.ascontiguousarray(proj_w.astype(bf16).T)
    # the k bias cancels in the softmax; the v bias passes through the
    # convex combination (weights sum to 1) and folds into the proj bias
    bv = qkv_b[2 * DIM:3 * DIM]
    bpv = (np.asarray(proj_b, np.float32) + proj_w @ bv).astype(bf16).reshape(1, -1)

    nbr = np.asarray(index_1).reshape(N, K).astype(np.int32)

    in_maps = []
    for c in range(N_CORES):
        c0 = c * n_own
        fown = featsT[:, c0:c0 + NOWN_PAD]
        if fown.shape[1] < NOWN_PAD:
            fown = np.concatenate(
                [fown, np.zeros((DIM, NOWN_PAD - fown.shape[1]), dtype=bf16)],
                axis=1)
        fown = np.ascontiguousarray(fown)
        nb = np.zeros((NOWN_PAD, K), dtype=np.int32)
        end = min(c0 + NOWN_PAD, N)
        nb[: end - c0] = nbr[c0:end]
        idx_host = np.ascontiguousarray(nb.reshape(TO, P, K).transpose(1, 0, 2))
        in_maps.append({
            "featsT": featsT, "fownT": fown,
            "wkvT": wkvT, "wqT": wqT, "bq": bqv,
            "wpT": wpT, "bp": bpv, "idx": idx_host,
        })
    return in_maps, n_own


def kernel(feats, xyz, index_0, index_1, index_0_offsets, n_max,
           qkv_w, qkv_b, proj_w, proj_b, _trace=False):
    global LAST_EXEC_NS
    N = feats.shape[0]
    n_own = N // N_CORES

    key = (N, n_own)
    if key not in _PROGRAM_CACHE:
        _PROGRAM_CACHE[key] = _build(N, n_own)
    nc = _PROGRAM_CACHE[key]

    in_maps, n_own = _host_prep(feats, index_1, qkv_w, qkv_b, proj_w, proj_b)
    try:
        res = run_bass_kernel_spmd(nc, in_maps, core_ids=list(range(N_CORES)),
                                   trace=_trace)
    except Exception:
        if not _trace:
            raise
        res = run_bass_kernel_spmd(nc, in_maps, core_ids=list(range(N_CORES)),
                                   trace=False)
    LAST_EXEC_NS = res.exec_time_ns
    outs = [np.asarray(res.results[c]["out"])[:n_own] for c in range(N_CORES)]
    return np.concatenate(outs, axis=0).astype(np.float32)

